# revision 11
# baseline (speedup 1.0000x reference)
"""Trainium2 Bass kernel for nn_DiagLRConv (diag-embedded 5x5 conv, pad=2).

Math: out[n,o,h,w] = sum_{i,k} filter_w[o,i,k] * x[n,i,h+k-2,w+k-2]

Scheme (per NeuronCore, 2 images, 8 cores data-parallel over batch):
  Rows grouped into 8-row phase blocks h = 8q + phi'.  The moving tile for
  block q holds, on partition (phi, i), the DIAGONALLY SHIFTED padded row
      T_q[phi,i][b] = Xp[i, 8q+phi, phi+b-8]
  so the 5-tap diagonal conv is a pure partition-band contraction:
      out[8q+phi', w] = sum_{i,k} w[o,i,k] * T_q[phi'+k, i][w - phi' + 8]
  Per block: one full-width K=128 matmul (band stationary S_main) plus a
  K=64 straddle matmul (S_next over the next tile's phases 0..3)
  accumulating into psum partitions 64:128.  IMAGE PARITY balances the PE:
  image 0 stores phases 0..3 in partitions 0:64, image 1 in 64:128, so the
  two images' half-width straddle matmuls run on OPPOSITE PE row halves
  into different psum banks -> they stream concurrently.  Steady state:
  3x512 PE cycles per 2 blocks (two full mains + one concurrent straddle
  pair) instead of 4x512.  The per-phase psum column shift (c = w - phi')
  is absorbed by the output DMA layout; the left image edge (w < phi')
  comes from batched 8-column matmuls into per-chunk-per-image edge banks.
  Host pre-builds the diag-shifted layout xs[(phi,i), q, b] so every DMA
  is a plain leading-partition slice with contiguous per-partition runs.
  fp16 in / fp16 out (fp32 psum accumulate); host does the final fp32 cast.
"""

import numpy as np

F16 = np.float16

_COMPILED = {}

B = 528          # buffer width  (b = 8 + (w - phi'); covers w-phi' in [-8, 520))
SW = 520         # staging/output width (s = w - phi' + 8 in [0, 520))
UW = 536         # host padded-row width (u = phi + b)


def _trace_nc(H):
    import concourse.mybir as mybir
    import concourse.tile as tile
    from concourse import bacc

    F32 = mybir.dt.float32
    FP16 = mybir.dt.float16

    assert H % 64 == 0
    NQ = H // 8              # 8-row output blocks per image
    CQ = 16 if NQ % 16 == 0 else 8
    NCH = NQ // CQ           # chunks of CQ blocks

    nc = bacc.Bacc(None, target_bir_lowering=False, debug=False)
    xs = nc.declare_dram_parameter("xs", [2, 128, NQ + 1, B], FP16, isOutput=False)
    wm = nc.declare_dram_parameter("wm", [128, 2, 128], FP16, isOutput=False)
    wn = nc.declare_dram_parameter("wn", [128, 64], FP16, isOutput=False)
    ys = nc.declare_dram_parameter("ys", [2, 128, NQ, SW], FP16, isOutput=True)

    LO, HI = slice(0, 64), slice(64, 128)

    with tile.TileContext(nc) as tc:
        with (
            tc.tile_pool(name="const", bufs=1) as const,
            tc.tile_pool(name="xpool", bufs=6) as xpool,
            tc.tile_pool(name="psum", bufs=6, space="PSUM") as psum,
            tc.tile_pool(name="pepool", bufs=2, space="PSUM") as pepool,
            tc.tile_pool(name="stpool", bufs=3) as stpool,
        ):
            xq = {}

            def load(c):
                if c >= NCH:
                    return
                nt = CQ if c < NCH - 1 else CQ + 1
                for m in range(2):
                    t = xpool.tile([128, nt, B], FP16, tag="xq", name=f"xq{m}_{c}")
                    nc.sync.dma_start(out=t[:], in_=xs[m, :, CQ * c : CQ * c + nt, :])
                    xq[(m, c)] = t

            rowh = [LO, HI]          # straddle row half per image
            load(0)
            wm_t = const.tile([128, 2, 128], FP16)
            nc.sync.dma_start(out=wm_t[:], in_=wm[:])
            wn_t = const.tile([128, 64], FP16)
            nc.sync.dma_start(out=wn_t[:], in_=wn[:])
            load(1)
            for c in range(NCH):
                load(c + 2)
                last = c == NCH - 1
                cur = [xq[(0, c)], xq[(1, c)]]
                sts, pes = [], []
                for m in range(2):
                    sts.append(stpool.tile([128, CQ, SW], FP16, tag="st",
                                           name=f"st{m}_{c}"))
                    pes.append(pepool.tile([128, 512], F32, tag="pe",
                                           name=f"pe{m}_{c}"))
                for qq in range(CQ):
                    if qq < CQ - 1 or last:
                        nxt, nq = cur, qq + 1
                    else:
                        nxt, nq = [xq[(0, c + 1)], xq[(1, c + 1)]], 0
                    pss = []
                    for m in range(2):
                        ps = psum.tile([128, 512], F32, tag="ps",
                                       name=f"ps{m}_{c}_{qq}")
                        pss.append(ps)
                        nc.tensor.matmul(
                            ps[:], wm_t[:, m, :], cur[m][:, qq, 8:520],
                            start=True, stop=False,
                        )
                    for m in range(2):
                        rh = rowh[m]
                        nc.tensor.matmul(
                            pss[m][64:128, :], wn_t[rh, :],
                            nxt[m][rh, nq, 16:528],
                            start=False, stop=True,
                            tile_position=(rh.start, 64),
                            skip_group_check=True,
                        )
                    for m in range(2):
                        eng = (nc.vector.tensor_copy if (qq + m) % 2 == 0
                               else nc.scalar.copy)
                        eng(sts[m][:, qq, 8:SW], pss[m][:])
                # batched edge matmuls (left image edge, w < phi')
                EW = 8 * CQ
                for m in range(2):
                    pe = pes[m]
                    rh = rowh[m]
                    nc.tensor.matmul(
                        pe[:, 0:EW], wm_t[:, m, :], cur[m][:, 0:CQ, 0:8],
                        start=True, stop=False,
                    )
                    if last:
                        nc.tensor.matmul(
                            pe[64:128, 0:EW], wn_t[rh, :],
                            cur[m][rh, 1 : CQ + 1, 8:16],
                            start=False, stop=True,
                            tile_position=(rh.start, 64), skip_group_check=True,
                        )
                    else:
                        nc.tensor.matmul(
                            pe[64:128, 0 : EW - 8], wn_t[rh, :],
                            cur[m][rh, 1:CQ, 8:16],
                            start=False, stop=False,
                            tile_position=(rh.start, 64), skip_group_check=True,
                        )
                        nc.tensor.matmul(
                            pe[64:128, EW - 8 : EW], wn_t[rh, :],
                            xq[(m, c + 1)][rh, 0, 8:16],
                            start=False, stop=True,
                            tile_position=(rh.start, 64), skip_group_check=True,
                        )
                    eng = nc.scalar.copy if m == 0 else nc.vector.tensor_copy
                    eng(sts[m][:, :, 0:8], pe[:, 0:EW])
                    if last:
                        h = CQ // 2
                        nc.sync.dma_start(
                            out=ys[m, :, CQ * c : CQ * c + h, :],
                            in_=sts[m][:, 0:h, :])
                        nc.sync.dma_start(
                            out=ys[m, :, CQ * c + h : CQ * c + CQ, :],
                            in_=sts[m][:, h:CQ, :])
                    else:
                        nc.sync.dma_start(out=ys[m, :, CQ * c : CQ * c + CQ, :],
                                          in_=sts[m][:])
    nc.compile()
    return nc


def _get_nc(H):
    if H not in _COMPILED:
        _COMPILED[H] = _trace_nc(H)
    return _COMPILED[H]


def _swap_rows(a):
    """Swap partition halves of a [128, ...] stationary (phase roll by 4)."""
    out = np.empty_like(a)
    out[0:64] = a[64:128]
    out[64:128] = a[0:64]
    return out


def _prep_inputs(x, filter_w, H):
    """x: [N,16,H,512] fp32, filter_w: [16,16,5] fp32 -> per-core in_maps."""
    N = x.shape[0]
    n_cores = N // 2
    NQ = H // 8
    x16 = x.astype(F16)
    fwT = filter_w.astype(F16).transpose(1, 0, 2)   # [i, o, k]

    wm0 = np.zeros((128, 128), dtype=F16)
    for k in range(5):
        for p in range(8 - k):
            wm0[16 * (p + k) : 16 * (p + k) + 16, 16 * p : 16 * p + 16] = fwT[:, :, k]
    wm = np.stack([wm0, _swap_rows(wm0)], axis=1)   # [128, 2, 128]
    wn = np.zeros((128, 64), dtype=F16)
    for pp in range(4, 8):
        for j in range(4):
            k = j + 8 - pp
            if k <= 4:
                wn[16 * j : 16 * j + 16,
                   16 * (pp - 4) : 16 * (pp - 4) + 16] = fwT[:, :, k]
    wn[64:128] = wn[0:64]

    in_maps = []
    for cid in range(n_cores):
        xpw = np.zeros((2, 16, 8 * (NQ + 1), UW), dtype=F16)
        xpw[:, :, 2 : H + 2, 10 : 522] = x16[2 * cid : 2 * cid + 2]
        xsv = np.empty((2, 8, 16, NQ + 1, B), dtype=F16)
        for phi in range(8):
            xsv[:, phi] = xpw[:, :, phi::8, phi : phi + B]
        xsv[1] = np.roll(xsv[1], 4, axis=0)          # image-1 half swap
        in_maps.append({"xs": xsv.reshape(2, 128, NQ + 1, B), "wm": wm, "wn": wn})
    return in_maps


def _reassemble(yk, H):
    """yk [2,128,NQ,SW] fp16 -> [2,16,H,512] fp32; h = 8q + phi, w = s + phi - 8."""
    NQ = H // 8
    ys = yk.reshape(2, 8, 16, NQ, SW)
    out = np.empty((2, 16, H, 512), dtype=np.float32)
    for phi in range(8):
        out[:, :, phi::8, :] = ys[:, phi, :, :, 8 - phi : 520 - phi]
    return out


def kernel(x, filter_w):
    from concourse.bass_utils import run_bass_kernel_spmd

    x = np.asarray(x)
    filter_w = np.asarray(filter_w)
    N, C, H, W = x.shape
    assert (C, W) == (16, 512) and N % 2 == 0

    nc = _get_nc(H)
    in_maps = _prep_inputs(x, filter_w, H)
    n_cores = len(in_maps)
    res = run_bass_kernel_spmd(nc, in_maps, list(range(n_cores)))
    out = np.empty((N, 16, H, 512), dtype=np.float32)
    for cid in range(n_cores):
        out[2 * cid : 2 * cid + 2] = _reassemble(res.results[cid]["ys"], H)
    return out


if __name__ == "__main__":
    import sys
    H = int(sys.argv[1]) if len(sys.argv) > 1 else 64
    rng = np.random.default_rng(0)
    x = rng.standard_normal((16, 16, H, 512)).astype(np.float32)
    fw = (rng.standard_normal((16, 16, 5)) * 0.1).astype(np.float32)
    out = kernel(x, fw)

    xpad = np.zeros((16, 16, H + 4, 516), dtype=np.float64)
    xpad[:, :, 2 : H + 2, 2:514] = x
    ref = np.zeros_like(out, dtype=np.float64)
    for k in range(5):
        sh = xpad[:, :, k : k + H, k : k + 512]
        ref += np.einsum("oik,nihw->nohw", fw[:, :, k : k + 1].astype(np.float64), sh)
    rel = np.linalg.norm(out - ref) / np.linalg.norm(ref)
    mx = np.abs(out - ref).max() / np.abs(ref).max()
    print(f"self-test H={H}: rel l2 err {rel:.3e}, max err {mx:.3e}")


# revision 12
# speedup vs baseline: 1.2547x; 1.2547x over previous
"""Trainium2 Bass kernel for nn_DiagLRConv (diag-embedded 5x5 conv, pad=2).

Math: out[n,o,h,w] = sum_{i,k} filter_w[o,i,k] * x[n,i,h+k-2,w+k-2]

Scheme (per NeuronCore, 2 images, 8 cores data-parallel over batch):
  Rows grouped into 8-row phase blocks h = 8q + phi'.  The moving tile for
  block q holds, on partition (phi, i), the DIAGONALLY SHIFTED padded row
      T_q[phi,i][b] = Xp[i, 8q+phi, phi+b-8]
  so the 5-tap diagonal conv is a pure partition-band contraction:
      out[8q+phi', w] = sum_{i,k} w[o,i,k] * T_q[phi'+k, i][w - phi' + 8]
  Per block: one full-width K=128 matmul (band stationary S_main) plus a
  K=64 straddle matmul (S_next over the next tile's phases 0..3)
  accumulating into psum partitions 64:128.  IMAGE PARITY balances the PE:
  image 0 stores phases 0..3 in partitions 0:64, image 1 in 64:128, so the
  two images' half-width straddle matmuls run on OPPOSITE PE row halves
  into different psum banks -> they stream concurrently.  Steady state:
  3x512 PE cycles per 2 blocks (two full mains + one concurrent straddle
  pair) instead of 4x512.  The per-phase psum column shift (c = w - phi')
  is absorbed by the output DMA layout; the left image edge (w < phi')
  comes from batched 8-column matmuls into per-chunk-per-image edge banks.
  Host pre-builds the diag-shifted layout xs[(phi,i), q, b] so every DMA
  is a plain leading-partition slice with contiguous per-partition runs.
  fp16 in / fp16 out (fp32 psum accumulate); host does the final fp32 cast.
"""

import numpy as np

F16 = np.float16

_COMPILED = {}

B = 528          # buffer width  (b = 8 + (w - phi'); covers w-phi' in [-8, 520))
SW = 520         # staging/output width (s = w - phi' + 8 in [0, 520))
UW = 536         # host padded-row width (u = phi + b)


def _trace_nc(H):
    import concourse.mybir as mybir
    import concourse.tile as tile
    from concourse import bacc

    F32 = mybir.dt.float32
    FP16 = mybir.dt.float16

    assert H % 64 == 0
    NQ = H // 8              # 8-row output blocks per image
    NCH = NQ // 8            # chunks of 8 blocks

    nc = bacc.Bacc(None, target_bir_lowering=False, debug=False)
    xs = nc.declare_dram_parameter("xs", [2, 128, NQ + 1, B], FP16, isOutput=False)
    wm = nc.declare_dram_parameter("wm", [128, 2, 128], FP16, isOutput=False)
    wn = nc.declare_dram_parameter("wn", [128, 64], FP16, isOutput=False)
    ys = nc.declare_dram_parameter("ys", [2, 128, NQ, SW], FP16, isOutput=True)

    LO, HI = slice(0, 64), slice(64, 128)

    with tile.TileContext(nc) as tc:
        with (
            tc.tile_pool(name="const", bufs=1) as const,
            tc.tile_pool(name="xpool", bufs=8) as xpool,
            tc.tile_pool(name="psum", bufs=6, space="PSUM") as psum,
            tc.tile_pool(name="pepool", bufs=2, space="PSUM") as pepool,
            tc.tile_pool(name="stpool", bufs=6) as stpool,
        ):
            xq = {}

            def load(c):
                if c >= NCH:
                    return
                nt = 8 if c < NCH - 1 else 9
                for m in range(2):
                    t = xpool.tile([128, nt, B], FP16, tag="xq", name=f"xq{m}_{c}")
                    nc.sync.dma_start(out=t[:], in_=xs[m, :, 8 * c : 8 * c + nt, :])
                    xq[(m, c)] = t

            rowh = [LO, HI]          # straddle row half per image
            load(0)
            wm_t = const.tile([128, 2, 128], FP16)
            nc.sync.dma_start(out=wm_t[:], in_=wm[:])
            wn_t = const.tile([128, 64], FP16)
            nc.sync.dma_start(out=wn_t[:], in_=wn[:])
            load(1)
            load(2)
            for c in range(NCH):
                load(c + 3)
                last = c == NCH - 1
                cur = [xq[(0, c)], xq[(1, c)]]
                sts, pes = [], []
                for m in range(2):
                    sts.append(stpool.tile([128, 8, SW], FP16, tag="st",
                                           name=f"st{m}_{c}"))
                    pes.append(pepool.tile([128, 8, 64], F32, tag="pe",
                                           name=f"pe{m}_{c}"))
                for qq in range(8):
                    if qq < 7 or last:
                        nxt, nq = cur, qq + 1
                    else:
                        nxt, nq = [xq[(0, c + 1)], xq[(1, c + 1)]], 0
                    pss = []
                    for m in range(2):
                        ps = psum.tile([128, 512], F32, tag="ps",
                                       name=f"ps{m}_{c}_{qq}")
                        pss.append(ps)
                        nc.tensor.matmul(
                            ps[:], wm_t[:, m, :], cur[m][:, qq, 8:520],
                            start=True, stop=False,
                        )
                    for m in range(2):
                        rh = rowh[m]
                        nc.tensor.matmul(
                            pss[m][64:128, :], wn_t[rh, :],
                            nxt[m][rh, nq, 16:528],
                            start=False, stop=True,
                            tile_position=(rh.start, 64),
                            skip_group_check=True,
                        )
                    for m in range(2):
                        eng = (nc.vector.tensor_copy if (qq + m) % 2 == 0
                               else nc.scalar.copy)
                        eng(sts[m][:, qq, 8:SW], pss[m][:])
                # batched edge matmuls (left image edge, w < phi')
                for m in range(2):
                    pe = pes[m]
                    rh = rowh[m]
                    nc.tensor.matmul(
                        pe[:, :, 0:8], wm_t[:, m, :], cur[m][:, 0:8, 0:8],
                        start=True, stop=False,
                    )
                    if last:
                        nc.tensor.matmul(
                            pe[64:128, 0:8, 0:8], wn_t[rh, :],
                            cur[m][rh, 1:9, 8:16],
                            start=False, stop=True,
                            tile_position=(rh.start, 64), skip_group_check=True,
                        )
                    else:
                        nc.tensor.matmul(
                            pe[64:128, 0:7, 0:8], wn_t[rh, :],
                            cur[m][rh, 1:8, 8:16],
                            start=False, stop=False,
                            tile_position=(rh.start, 64), skip_group_check=True,
                        )
                        nc.tensor.matmul(
                            pe[64:128, 7, 0:8], wn_t[rh, :],
                            xq[(m, c + 1)][rh, 0, 8:16],
                            start=False, stop=True,
                            tile_position=(rh.start, 64), skip_group_check=True,
                        )
                    eng = nc.scalar.copy if m == 0 else nc.vector.tensor_copy
                    eng(sts[m][:, :, 0:8], pe[:, :, 0:8])
                    if last:
                        nc.sync.dma_start(out=ys[m, :, 8 * c : 8 * c + 4, :],
                                          in_=sts[m][:, 0:4, :])
                        nc.sync.dma_start(out=ys[m, :, 8 * c + 4 : 8 * c + 8, :],
                                          in_=sts[m][:, 4:8, :])
                    else:
                        nc.sync.dma_start(out=ys[m, :, 8 * c : 8 * c + 8, :],
                                          in_=sts[m][:])
    nc.compile()
    return nc


def _get_nc(H):
    if H not in _COMPILED:
        _COMPILED[H] = _trace_nc(H)
    return _COMPILED[H]


def _swap_rows(a):
    """Swap partition halves of a [128, ...] stationary (phase roll by 4)."""
    out = np.empty_like(a)
    out[0:64] = a[64:128]
    out[64:128] = a[0:64]
    return out


def _prep_inputs(x, filter_w, H):
    """x: [N,16,H,512] fp32, filter_w: [16,16,5] fp32 -> per-core in_maps."""
    N = x.shape[0]
    n_cores = N // 2
    NQ = H // 8
    x16 = x.astype(F16)
    fwT = filter_w.astype(F16).transpose(1, 0, 2)   # [i, o, k]

    wm0 = np.zeros((128, 128), dtype=F16)
    for k in range(5):
        for p in range(8 - k):
            wm0[16 * (p + k) : 16 * (p + k) + 16, 16 * p : 16 * p + 16] = fwT[:, :, k]
    wm = np.stack([wm0, _swap_rows(wm0)], axis=1)   # [128, 2, 128]
    wn = np.zeros((128, 64), dtype=F16)
    for pp in range(4, 8):
        for j in range(4):
            k = j + 8 - pp
            if k <= 4:
                wn[16 * j : 16 * j + 16,
                   16 * (pp - 4) : 16 * (pp - 4) + 16] = fwT[:, :, k]
    wn[64:128] = wn[0:64]

    in_maps = []
    for cid in range(n_cores):
        xpw = np.zeros((2, 16, 8 * (NQ + 1), UW), dtype=F16)
        xpw[:, :, 2 : H + 2, 10 : 522] = x16[2 * cid : 2 * cid + 2]
        xsv = np.empty((2, 8, 16, NQ + 1, B), dtype=F16)
        for phi in range(8):
            xsv[:, phi] = xpw[:, :, phi::8, phi : phi + B]
        xsv[1] = np.roll(xsv[1], 4, axis=0)          # image-1 half swap
        in_maps.append({"xs": xsv.reshape(2, 128, NQ + 1, B), "wm": wm, "wn": wn})
    return in_maps


def _reassemble(yk, H):
    """yk [2,128,NQ,SW] fp16 -> [2,16,H,512] fp32; h = 8q + phi, w = s + phi - 8."""
    NQ = H // 8
    ys = yk.reshape(2, 8, 16, NQ, SW)
    out = np.empty((2, 16, H, 512), dtype=np.float32)
    for phi in range(8):
        out[:, :, phi::8, :] = ys[:, phi, :, :, 8 - phi : 520 - phi]
    return out


def kernel(x, filter_w):
    from concourse.bass_utils import run_bass_kernel_spmd

    x = np.asarray(x)
    filter_w = np.asarray(filter_w)
    N, C, H, W = x.shape
    assert (C, W) == (16, 512) and N % 2 == 0

    nc = _get_nc(H)
    in_maps = _prep_inputs(x, filter_w, H)
    n_cores = len(in_maps)
    res = run_bass_kernel_spmd(nc, in_maps, list(range(n_cores)))
    out = np.empty((N, 16, H, 512), dtype=np.float32)
    for cid in range(n_cores):
        out[2 * cid : 2 * cid + 2] = _reassemble(res.results[cid]["ys"], H)
    return out


if __name__ == "__main__":
    import sys
    H = int(sys.argv[1]) if len(sys.argv) > 1 else 64
    rng = np.random.default_rng(0)
    x = rng.standard_normal((16, 16, H, 512)).astype(np.float32)
    fw = (rng.standard_normal((16, 16, 5)) * 0.1).astype(np.float32)
    out = kernel(x, fw)

    xpad = np.zeros((16, 16, H + 4, 516), dtype=np.float64)
    xpad[:, :, 2 : H + 2, 2:514] = x
    ref = np.zeros_like(out, dtype=np.float64)
    for k in range(5):
        sh = xpad[:, :, k : k + H, k : k + 512]
        ref += np.einsum("oik,nihw->nohw", fw[:, :, k : k + 1].astype(np.float64), sh)
    rel = np.linalg.norm(out - ref) / np.linalg.norm(ref)
    mx = np.abs(out - ref).max() / np.abs(ref).max()
    print(f"self-test H={H}: rel l2 err {rel:.3e}, max err {mx:.3e}")
